# revision 5
# baseline (speedup 1.0000x reference)
"""MoE MLP (GPT-2 style experts, top-2 routing) on 8 Trainium2 NeuronCores.

Strategy (expert-parallel, per sharding hint):
  - Host: router matmul + softmax + top-2 + renormalize, then dispatch tokens
    by expert id -> per-core gathered token block, transposed to [C, M].
    Within each expert block, tokens routed here as their SECONDARY expert
    (combine gate <= 0.5) come first.
  - Device (core e): outT = w_proj[e].T @ gelu(w_fc[e].T @ xT + b_fc[e]) as
    tiled PE matmuls, fp16 operands with f32 PSUM accumulation. Both weight
    matrices stay resident in SBUF.
  - Hybrid precision: an error-budgeted slice of the contraction runs as
    fp8e4 DoubleRow matmuls (2 fp8 weights/PE cell -> 2x throughput; measured
    on-device rel_l2 of full-fp8 is 5.4e-2, and error scales with
    sqrt(fp8 work fraction) plus the gate weighting of the affected tokens).
    fc: k-subtiles 0..3 on the first (secondary-token) m-tile; proj:
    ff-subtiles 0..3 on all m-tiles. Operands are pre-scaled (x*16, w*32) so
    the fp8 and fp16 terms accumulate into one PSUM group at a common scale;
    dequant folds into the ACT gelu scale and the host combine.
  - Host: combine: out[tok] += gate * (y + b_proj[e]) for each routed pair.
"""

import functools
import os

import numpy as np
import ml_dtypes

import concourse.bacc as bacc
import concourse.mybir as mybir
import concourse.tile as tile
from concourse.bass_utils import run_bass_kernel_spmd

N_EMBD = 1024
D_FF = 4096
N_EXPERTS = 8
TOP_K = 2
N_CORES = 8
P = 128
KT = N_EMBD // P      # 8 k-tiles (contraction over n_embd)
FT = D_FF // P        # 32 ff-tiles (contraction over d_ff for proj)
CT = N_EMBD // P      # 8 output-channel tiles
MSZ = 512             # moving (token) tile width

DT16 = mybir.dt.float16
FP8 = mybir.dt.float8e4
F32 = mybir.dt.float32
FP8_NP = ml_dtypes.float8_e4m3
DR = mybir.MatmulPerfMode.DoubleRow

SX = 16.0   # x pre-scale (f16 and fp8 copies)
SW = 32.0   # w_fc pre-scale
SP = 32.0   # w_proj pre-scale (divided out on host)

# Hybrid-precision knobs (error budget: full-fp8 measures 5.4e-2 rel_l2;
# fraction f of contraction in fp8 -> ~3.8e-2*sqrt(f_fc+f_pj), secondary-token
# tiles further weighted by gate^2 ~ 0.45):
FC_DR_TILES = 1   # m-tiles (secondary tokens first) with fc k0..3 in fp8 DR
PJ_DR_PAIRS = 2   # leading ff-subtile pairs of proj in fp8 DR on all m-tiles


def _m_tiles(M, msz=MSZ):
    out = []
    m0 = 0
    while m0 < M:
        out.append((m0, min(msz, M - m0)))
        m0 += msz
    return out


@functools.lru_cache(maxsize=8)
def _build(M, repeat=1, fc_dr_tiles=FC_DR_TILES, pj_dr_pairs=PJ_DR_PAIRS,
           msz=MSZ, psa_bufs=3, psb_bufs=3, x_bufs=2, h_bufs=1, o_bufs=3):
    """Bass program: per-core dense expert MLP over M gathered tokens."""
    nc = bacc.Bacc("TRN2", target_bir_lowering=False, debug=False)

    xT = nc.dram_tensor("xT", [KT, P, M], DT16, kind="ExternalInput")
    wfc = nc.dram_tensor("w_fc", [KT, P, D_FF], DT16, kind="ExternalInput")
    bfcT = nc.dram_tensor("b_fcT", [P, FT], F32, kind="ExternalInput")
    wproj = nc.dram_tensor("w_proj", [FT, P, N_EMBD], DT16,
                           kind="ExternalInput")
    outT = nc.dram_tensor("outT", [CT, P, M], DT16, kind="ExternalOutput")
    if fc_dr_tiles:
        x8T = nc.dram_tensor("x8T", [4, P, M], FP8, kind="ExternalInput")
        wfc8 = nc.dram_tensor("w_fc8", [4, P, D_FF], FP8,
                              kind="ExternalInput")
    if pj_dr_pairs:
        wproj8 = nc.dram_tensor("w_proj8", [2 * pj_dr_pairs, P, N_EMBD], FP8,
                                kind="ExternalInput")

    with tile.TileContext(nc) as tc:
        with tc.tile_pool(name="weights", bufs=1) as wpool, \
             tc.tile_pool(name="xp", bufs=x_bufs) as xpool, \
             tc.tile_pool(name="hp", bufs=h_bufs) as hpool, \
             tc.tile_pool(name="h8p", bufs=2) as h8pool, \
             tc.tile_pool(name="op", bufs=o_bufs) as opool, \
             tc.tile_pool(name="psA", bufs=psa_bufs, space="PSUM") as psA, \
             tc.tile_pool(name="psB", bufs=psb_bufs, space="PSUM") as psB:

            def load_x(m0, mw, want8):
                x_sb = xpool.tile([P, KT, msz], DT16, tag="x", name="x_sb")
                for k in range(KT):
                    nc.sync.dma_start(x_sb[:, k, :mw], xT[k, :, m0:m0 + mw])
                x8_sb = None
                if want8:
                    x8_sb = xpool.tile([P, 4, msz], FP8, tag="x8",
                                       name="x8_sb")
                    for k in range(4):
                        nc.sync.dma_start(x8_sb[:, k, :mw],
                                          x8T[k, :, m0:m0 + mw])
                return x_sb, x8_sb

            tiles = _m_tiles(M, msz)
            pre_x = load_x(tiles[0][0], tiles[0][1], fc_dr_tiles > 0)

            def load_weights():
                wfc_sb = wpool.tile([P, KT, D_FF], DT16, tag="wfc",
                                    name="wfc_sb")
                CHUNK = 1024
                for c0 in range(0, D_FF, CHUNK):
                    for k in range(KT):
                        nc.sync.dma_start(
                            wfc_sb[:, k, c0:c0 + CHUNK],
                            wfc[k, :, c0:c0 + CHUNK]
                        )
                bfc_sb = wpool.tile([P, FT], F32, tag="bfc", name="bfc_sb")
                nc.sync.dma_start(bfc_sb[:, :], bfcT[:, :])
                wproj_sb = wpool.tile([P, FT, N_EMBD], DT16, tag="wproj",
                                      name="wproj_sb")
                for f in range(FT):
                    nc.sync.dma_start(wproj_sb[:, f, :], wproj[f, :, :])
                wfc8_sb = wproj8_sb = None
                if fc_dr_tiles:
                    wfc8_sb = wpool.tile([P, 4, D_FF], FP8, tag="wfc8",
                                         name="wfc8_sb")
                    for k in range(4):
                        nc.sync.dma_start(wfc8_sb[:, k, :], wfc8[k, :, :])
                if pj_dr_pairs:
                    wproj8_sb = wpool.tile([P, 2 * pj_dr_pairs, N_EMBD], FP8,
                                           tag="wproj8", name="wproj8_sb")
                    for f in range(2 * pj_dr_pairs):
                        nc.sync.dma_start(wproj8_sb[:, f, :], wproj8[f, :, :])
                return wfc_sb, bfc_sb, wproj_sb, wfc8_sb, wproj8_sb

            wfc_sb, bfc_sb, wproj_sb, wfc8_sb, wproj8_sb = load_weights()

            for _r in range(repeat):
                for ti, (m0, mw) in enumerate(tiles):
                    fc_dr = ti < fc_dr_tiles
                    if _r == 0 and ti == 0:
                        x_sb, x8_sb = pre_x
                    else:
                        x_sb, x8_sb = load_x(m0, mw, fc_dr)

                    hT_sb = hpool.tile([P, FT, msz], DT16, tag="h",
                                       name="hT_sb")
                    if pj_dr_pairs:
                        h8_sb = h8pool.tile([P, 2 * pj_dr_pairs, msz], FP8,
                                            tag="h8", name="h8_sb")
                    for f in range(FT):
                        fsl = slice(f * P, (f + 1) * P)
                        ps = psA.tile([P, msz], F32, tag="psA", name="ps_fc")
                        k0 = 4 if fc_dr else 0
                        if fc_dr:
                            for kk in (0, 2):
                                nc.tensor.matmul(
                                    ps[:, :mw],
                                    wfc8_sb[:, kk:kk + 2, fsl],
                                    x8_sb[:, kk:kk + 2, :mw],
                                    start=(kk == 0), stop=False,
                                    perf_mode=DR,
                                )
                        for k in range(k0, KT):
                            nc.tensor.matmul(
                                ps[:, :mw],
                                wfc_sb[:, k, fsl],
                                x_sb[:, k, :mw],
                                start=(k == 0 and not fc_dr),
                                stop=(k == KT - 1),
                            )
                        nc.scalar.activation(
                            hT_sb[:, f, :mw], ps[:, :mw],
                            mybir.ActivationFunctionType.Gelu,
                            bias=bfc_sb[:, f:f + 1],
                            scale=1.0 / (SX * SW),
                        )
                        if pj_dr_pairs and f < 2 * pj_dr_pairs:
                            nc.vector.tensor_copy(h8_sb[:, f, :mw],
                                                  hT_sb[:, f, :mw])

                    for c in range(CT):
                        csl = slice(c * P, (c + 1) * P)
                        ps2 = psB.tile([P, msz], F32, tag="psB", name="ps_pj")
                        for pp in range(pj_dr_pairs):
                            nc.tensor.matmul(
                                ps2[:, :mw],
                                wproj8_sb[:, 2 * pp:2 * pp + 2, csl],
                                h8_sb[:, 2 * pp:2 * pp + 2, :mw],
                                start=(pp == 0), stop=False,
                                perf_mode=DR,
                            )
                        f0 = 2 * pj_dr_pairs
                        for f in range(f0, FT):
                            nc.tensor.matmul(
                                ps2[:, :mw],
                                wproj_sb[:, f, csl],
                                hT_sb[:, f, :mw],
                                start=(f == 0 and pj_dr_pairs == 0),
                                stop=(f == FT - 1),
                            )
                        o_sb = opool.tile([P, msz], DT16, tag="o", name="o_sb")
                        nc.vector.tensor_copy(o_sb[:, :mw], ps2[:, :mw])
                        nc.sync.dma_start(outT[c, :, m0:m0 + mw],
                                          o_sb[:, :mw])

    nc.compile()
    return nc


def _route(x_flat, router_w):
    """Top-2 routing, matching the reference numerics (f32)."""
    N = x_flat.shape[0]
    logits = x_flat @ router_w.T                      # [N, E]
    logits -= logits.max(axis=-1, keepdims=True)
    p = np.exp(logits)
    p /= p.sum(axis=-1, keepdims=True)
    rows = np.arange(N)
    i1 = p.argmax(axis=-1)
    p1 = p[rows, i1]
    pm = p.copy()
    pm[rows, i1] = -1.0
    i2 = pm.argmax(axis=-1)
    p2 = p[rows, i2]
    s = p1 + p2 + 1e-9
    return i1, i2, p1 / s, p2 / s


def _q8(a):
    return np.clip(a, -240.0, 240.0).astype(FP8_NP)


def prepare(x, router_w, w_fc, b_fc, w_proj, b_proj):
    """Host routing + per-core input maps. Returns (in_maps, state)."""
    x = np.asarray(x, dtype=np.float32)
    router_w = np.asarray(router_w, dtype=np.float32)
    w_fc = np.asarray(w_fc, dtype=np.float32)
    b_fc = np.asarray(b_fc, dtype=np.float32)
    w_proj = np.asarray(w_proj, dtype=np.float32)

    B, T, C = x.shape
    x_flat = x.reshape(-1, C)

    i1, i2, g1, g2 = _route(x_flat, router_w)

    idxs, gates = [], []
    for e in range(N_EXPERTS):
        # Secondary-routed tokens (gate <= 0.5) first: the fp8 m-tiles sit at
        # the front of the block where quantization error is gate-attenuated.
        sec = np.flatnonzero(i2 == e)
        pri = np.flatnonzero(i1 == e)
        idx = np.concatenate([sec, pri])
        g = np.concatenate([g2[sec], g1[pri]]).astype(np.float32)
        idxs.append(idx)
        gates.append(g)

    max_cnt = max(len(ix) for ix in idxs)
    M = max(P, ((max_cnt + P - 1) // P) * P)

    in_maps = []
    for e in range(N_EXPERTS):
        idx = idxs[e]
        xg = np.zeros((M, C), dtype=np.float32)
        xg[: len(idx)] = x_flat[idx]
        xsc = np.ascontiguousarray(xg.T) * SX
        m = {
            "xT": xsc.reshape(KT, P, M).astype(np.float16),
            "w_fc": (w_fc[e] * SW).reshape(KT, P, D_FF).astype(np.float16),
            "b_fcT": np.ascontiguousarray(b_fc[e].reshape(FT, P).T),
            "w_proj": (w_proj[e] * SP).reshape(FT, P, N_EMBD).astype(
                np.float16),
        }
        if FC_DR_TILES:
            m["x8T"] = _q8(xsc.reshape(KT, P, M)[:4])
            m["w_fc8"] = _q8((w_fc[e][:512] * SW).reshape(4, P, D_FF))
        if PJ_DR_PAIRS:
            m["w_proj8"] = _q8(
                (w_proj[e][:2 * PJ_DR_PAIRS * P] * SP)
                .reshape(2 * PJ_DR_PAIRS, P, N_EMBD))
        in_maps.append(m)
    state = (idxs, gates, M)
    return in_maps, state


def build(state, repeat=1):
    return _build(state[2], repeat)


def kernel(x, router_w, w_fc, b_fc, w_proj, b_proj):
    x = np.asarray(x, dtype=np.float32)
    b_proj = np.asarray(b_proj, dtype=np.float32)
    B, T, C = x.shape
    N = B * T

    in_maps, state = prepare(x, router_w, w_fc, b_fc, w_proj, b_proj)
    idxs, gates, M = state

    repeat = int(os.environ.get("MOE_KERNEL_REPEAT", "1"))
    nc = _build(M, repeat)

    res = run_bass_kernel_spmd(nc, in_maps, core_ids=list(range(N_CORES)))

    out_flat = np.zeros((N, C), dtype=np.float32)
    for e in range(N_EXPERTS):
        idx = idxs[e]
        yT = res.results[e]["outT"].reshape(C, M).astype(np.float32)  # SP-scaled
        y = yT.T[: len(idx)] * (1.0 / SP)                # [n_e, C]
        out_flat[idx] += gates[e][:, None] * (y + b_proj[e])

    return out_flat.reshape(B, T, C)
